# revision 2
# baseline (speedup 1.0000x reference)
"""Trainium2 Bass kernel for linear attention (silu+1 feature map, cumsum over T)
with dense 1024x1024 in/out projections.

Sharding: 8 cores = 4 batches x 2 head-groups (8 heads / 512 channels each).
Each core computes q/k/v projections for its 512 channels over the full
T=4096 of its batch, the linear-attention recurrence locally, and a partial
Wo projection (512 in-ch -> all 1024 out-ch). The host sums the two bf16
partials per batch, scales by 1/64 and adds bo. No cross-core traffic.

v2 engine balance (vs v1):
 - v-bias folded into the Wv matmul as a rank-1 ones-row update; phi_k*v
   reads the v PSUM directly (no ACT copy of v).
 - den: reciprocal reads the PSUM accumulator directly (eps dropped: den>=33),
   1/8 folded into the fm broadcast matrix (no ACT dens copy).
 - cumsums/phi_q/pq/nm run at FD=1024 (slab pairs) to amortize DVE init.
 - pq/nm multiplies and some scans moved to the (otherwise idle) GPSIMD/Pool
   engine; wo PSUM->SBUF copies stay on ACT (PSUM is unreachable from Pool).
 - output partials in bf16 (half the write traffic), input DMAs merged
   (1 per x slab, 1 per weight) to cut SP/HWDGE issue overhead.
"""

import numpy as np
import ml_dtypes

import concourse.bass as bass
import concourse.mybir as mybir
from concourse import bacc, tile
from concourse.bass_utils import run_bass_kernel_spmd

BF16 = mybir.dt.bfloat16
F32 = mybir.dt.float32
FP8 = mybir.dt.float8e4
DR = mybir.MatmulPerfMode.DoubleRow
XS = 0.125        # host scales x by XS, weights by 1/(XS*PS)
PS = 0.125        # ACT scale undoing the fp8 pre-scaling: psum*PS = true value
ADD = mybir.AluOpType.add
MULT = mybir.AluOpType.mult
BYPASS = mybir.AluOpType.bypass
SILU = mybir.ActivationFunctionType.Silu
COPY = mybir.ActivationFunctionType.Copy

B, C, T = 4, 1024, 4096
H, DH = 16, 64
CG = 512            # channels per head-group (per core)
S = 512             # projection slab (PSUM-bound)
P2 = 2 * S          # elementwise pair width
NPAIR = T // P2     # 4
NCH = CG // 128     # 4 chunks of 128 channels
KCH = C // 128      # 8 input-channel chunks
MO = C // 128       # 8 output-channel chunks

# which pair-level scans run on the Pool/GPSIMD engine (per chunk)
POOL_SCAN_KS = (True, True, False, False)
POOL_SCAN_KVS = (False, False, False, False)


def build():
    """Build the per-core Bass program (identical on all 8 cores)."""
    nc = bacc.Bacc(target_bir_lowering=False)

    x_d = nc.declare_dram_parameter("x", [C, T], FP8, isOutput=False)
    wq_d = nc.declare_dram_parameter("wq", [C, CG], FP8, isOutput=False)
    wk_d = nc.declare_dram_parameter("wk", [C, CG], FP8, isOutput=False)
    wv_d = nc.declare_dram_parameter("wv", [C, CG], FP8, isOutput=False)
    wo_d = nc.declare_dram_parameter("wo", [CG, C], FP8, isOutput=False)
    bq_d = nc.declare_dram_parameter("bq", [128, NCH], F32, isOutput=False)
    bv_d = nc.declare_dram_parameter("bv8", [1, CG], BF16, isOutput=False)
    em_d = nc.declare_dram_parameter("emat", [128, NCH, 8], BF16, isOutput=False)
    fm_d = nc.declare_dram_parameter("fmat", [8, CG], BF16, isOutput=False)
    on_d = nc.declare_dram_parameter("ones", [128, P2], BF16, isOutput=False)
    out_d = nc.declare_dram_parameter("out", [C, T], BF16, isOutput=True)

    with tile.TileContext(nc) as tc:
        from contextlib import ExitStack

        with ExitStack() as ctx:
            wpool = ctx.enter_context(tc.tile_pool(name="w", bufs=1))
            xpool = ctx.enter_context(tc.tile_pool(name="xp", bufs=3))
            ppool = ctx.enter_context(tc.tile_pool(name="proj", bufs=3, space="PSUM"))
            dpool = ctx.enter_context(tc.tile_pool(name="denp", bufs=1, space="PSUM"))
            bpool = ctx.enter_context(tc.tile_pool(name="bcast", bufs=2, space="PSUM"))
            opool = ctx.enter_context(tc.tile_pool(name="wops", bufs=2, space="PSUM"))
            apool = ctx.enter_context(tc.tile_pool(name="act", bufs=2))
            spool = ctx.enter_context(tc.tile_pool(name="state", bufs=2))
            outpool = ctx.enter_context(tc.tile_pool(name="outp", bufs=2))

            wq_t = wk_t = wv_t = wo_t = None
            em_t = fm_t = bq_t = bv_t = ones_t = None

            prev_ks = [None] * NCH
            prev_kvs = [None] * NCH

            def load_weights():
                nonlocal wq_t, wk_t, wv_t, wo_t, em_t, fm_t, bq_t, bv_t, ones_t

                def ld(shape, dt, src, tag):
                    t = wpool.tile(shape, dt, tag=tag, name=tag)
                    nc.sync.dma_start(t[:], src)
                    return t

                wq_t = ld([128, KCH, CG], FP8,
                          wq_d.rearrange("(ko ki) m -> ki ko m", ko=KCH), "wq")
                wk_t = ld([128, KCH, CG], FP8,
                          wk_d.rearrange("(ko ki) m -> ki ko m", ko=KCH), "wk")
                wv_t = ld([128, KCH, CG], FP8,
                          wv_d.rearrange("(ko ki) m -> ki ko m", ko=KCH), "wv")
                bq_t = ld([128, NCH], F32, bq_d[:, :], "bq")
                bv_t = ld([1, CG], BF16, bv_d[:, :], "bv8")
                ones_t = ld([128, P2], BF16, on_d[:, :], "ones")
                em_t = ld([128, NCH, 8], BF16, em_d[:, :, :], "em")
                fm_t = ld([8, CG], BF16, fm_d[:, :], "fm")
                wo_t = ld([128, NCH, C], FP8,
                          wo_d.rearrange("(ko ki) m -> ki ko m", ko=NCH), "wo")

            def stage_a(p, si, sq2, sk2, pk2):
                """One 512-token slab: x DMA, q/k/v projections, silu, pk."""
                t0 = P2 * p + S * si
                hs = slice(S * si, S * si + S)
                x_t = xpool.tile([128, KCH, S], FP8, tag="x", name=f"x{p}_{si}")
                nc.sync.dma_start(
                    x_t[:], x_d[:, t0:t0 + S].rearrange("(ko ki) t -> ki ko t", ko=KCH))
                if wq_t is None:
                    load_weights()
                K2 = KCH // 2
                for c in range(NCH):
                    cs = slice(128 * c, 128 * (c + 1))
                    ps_q = ppool.tile([128, S], F32, tag="proj", name=f"psq{p}{si}_{c}")
                    for k in range(K2):
                        nc.tensor.matmul(ps_q[:], wq_t[:, 2 * k:2 * k + 2, cs],
                                         x_t[:, 2 * k:2 * k + 2, :],
                                         start=(k == 0), stop=(k == K2 - 1), perf_mode=DR)
                    nc.scalar.activation(sq2[c][:, hs], ps_q[:], SILU,
                                         bias=bq_t[:, c:c + 1], scale=PS)
                    ps_k = ppool.tile([128, S], F32, tag="proj", name=f"psk{p}{si}_{c}")
                    for k in range(K2):
                        nc.tensor.matmul(ps_k[:], wk_t[:, 2 * k:2 * k + 2, cs],
                                         x_t[:, 2 * k:2 * k + 2, :],
                                         start=(k == 0), stop=(k == K2 - 1), perf_mode=DR)
                    nc.scalar.activation(sk2[c][:, hs], ps_k[:], SILU, scale=PS)
                    ps_v = ppool.tile([128, S], F32, tag="proj", name=f"psv{p}{si}_{c}")
                    # rank-1 ones-row update adds 8*bv into the raw v PSUM
                    nc.tensor.matmul(ps_v[:], bv_t[0:1, cs], ones_t[0:1, :S],
                                     start=True, stop=False)
                    for k in range(K2):
                        nc.tensor.matmul(ps_v[:], wv_t[:, 2 * k:2 * k + 2, cs],
                                         x_t[:, 2 * k:2 * k + 2, :],
                                         start=False, stop=(k == K2 - 1), perf_mode=DR)
                    # pk = (silu_k + 1) * (8*v)   [raw 8x scale, fixed via fm]
                    nc.vector.scalar_tensor_tensor(
                        pk2[c][:, hs], sk2[c][:, hs], 1.0, ps_v[:], op0=ADD, op1=MULT)

            def stage_scan(p, sq2, sk2, pk2):
                """Pair-level: phi_q, cumsums, pq, nm (DVE + Pool)."""
                phq2, pq2, nm2, ks2, kvs2 = [], [], [], [], []
                for c in range(NCH):
                    phq = apool.tile([128, P2], BF16, tag=f"phq{c}", name=f"phq{p}_{c}")
                    nc.vector.tensor_scalar_add(phq[:], sq2[c][:], 1.0)
                    ks = spool.tile([128, P2], BF16, tag=f"ks{c}", name=f"ks{p}_{c}")
                    ik = 0.0 if p == 0 else prev_ks[c][:, P2 - 1:P2]
                    eng_ks = nc.gpsimd if POOL_SCAN_KS[c] else nc.vector
                    eng_ks.tensor_tensor_scan(ks[:], sk2[c][:], ones_t[:, :],
                                              initial=ik, op0=ADD, op1=ADD)
                    kvs = spool.tile([128, P2], BF16, tag=f"kvs{c}", name=f"kvs{p}_{c}")
                    ikv = 0.0 if p == 0 else prev_kvs[c][:, P2 - 1:P2]
                    eng_kv = nc.gpsimd if POOL_SCAN_KVS[c] else nc.vector
                    eng_kv.tensor_tensor_scan(kvs[:], pk2[c][:], ones_t[:, :],
                                              initial=ikv, op0=ADD, op1=BYPASS)
                    prev_ks[c], prev_kvs[c] = ks, kvs
                    pq = apool.tile([128, P2], BF16, tag=f"pq{c}", name=f"pq{p}_{c}")
                    nc.gpsimd.tensor_mul(pq[:], phq[:], ks[:])
                    nm = apool.tile([128, P2], BF16, tag=f"nm{c}", name=f"nm{p}_{c}")
                    nc.gpsimd.tensor_mul(nm[:], phq[:], kvs[:])
                    phq2.append(phq), pq2.append(pq), nm2.append(nm)
                    ks2.append(ks), kvs2.append(kvs)
                return phq2, pq2, nm2, ks2, kvs2

            def tail_a(p, pq2):
                """den -> reciprocal -> bf16 rec for pair p."""
                rec2 = apool.tile([8, P2], BF16, tag="rec2", name=f"rec2{p}")
                for h in range(2):
                    hs = slice(S * h, S * h + S)
                    den_ps = dpool.tile([8, S], F32, tag="den", name=f"den{p}_{h}")
                    for c in range(NCH):
                        nc.tensor.matmul(den_ps[:], em_t[:, c, :], pq2[c][:, hs],
                                         start=(c == 0), stop=(c == NCH - 1))
                    rec32 = apool.tile([8, S], F32, tag="rec32", name=f"rec32{p}_{h}")
                    nc.vector.reciprocal_approx_fast(rec32[:], den_ps[:])
                    nc.scalar.copy(rec2[:, hs], rec32[:])
                return rec2

            def tail_b(p, nm2, rec2):
                """broadcast -> attn (fp8) -> Wo -> bf16 out DMA for pair p."""
                at_l = [outpool.tile([128, 2, P2], FP8, tag=f"at{kk}", name=f"at{p}_{kk}")
                        for kk in range(NCH // 2)]
                for c in range(NCH):
                    cs = slice(128 * c, 128 * (c + 1))
                    for h in range(2):
                        hs = slice(S * h, S * h + S)
                        rb = bpool.tile([128, S], F32, tag="rb", name=f"rb{p}_{c}{h}")
                        nc.tensor.matmul(rb[:], fm_t[:, cs], rec2[:, hs],
                                         start=True, stop=True)
                        nc.vector.tensor_mul(at_l[c // 2][:, c % 2, hs],
                                             nm2[c][:, hs], rb[:])
                for h in range(2):
                    tts = slice(P2 * p + S * h, P2 * p + S * h + S)
                    hs = slice(S * h, S * h + S)
                    for moo in range(MO // 2):
                        ot = outpool.tile([128, 2, S], BF16, tag=f"ot{moo}",
                                          name=f"ot{p}_{h}_{moo}")
                        for mo2 in range(2):
                            mo = 2 * moo + mo2
                            ms = slice(128 * mo, 128 * (mo + 1))
                            wo_ps = opool.tile([128, S], F32, tag="wo",
                                               name=f"wo{p}_{h}_{mo}")
                            for kk in range(NCH // 2):
                                nc.tensor.matmul(
                                    wo_ps[:], wo_t[:, 2 * kk:2 * kk + 2, ms],
                                    at_l[kk][:, :, hs],
                                    start=(kk == 0), stop=(kk == NCH // 2 - 1),
                                    perf_mode=DR)
                            nc.scalar.copy(ot[:, mo2, :], wo_ps[:])
                        nc.sync.dma_start(
                            out_d[256 * moo:256 * (moo + 1), tts].rearrange(
                                "(mo2 ki) t -> ki mo2 t", mo2=2),
                            ot[:])

            hist = {}
            for p in range(NPAIR):
                sq2 = [apool.tile([128, P2], BF16, tag=f"sq{c}", name=f"sq{p}_{c}")
                       for c in range(NCH)]
                sk2 = [apool.tile([128, P2], BF16, tag=f"sk{c}", name=f"sk{p}_{c}")
                       for c in range(NCH)]
                pk2 = [apool.tile([128, P2], BF16, tag=f"pk{c}", name=f"pk{p}_{c}")
                       for c in range(NCH)]
                stage_a(p, 0, sq2, sk2, pk2)
                stage_a(p, 1, sq2, sk2, pk2)
                if p >= 2:
                    nm2_o, rec2_o = hist[p - 2]
                    tail_b(p - 2, nm2_o, rec2_o)
                phq2, pq2, nm2, ks2, kvs2 = stage_scan(p, sq2, sk2, pk2)
                if p >= 1:
                    pq2_o, nm2_o = hist.pop(p - 1)
                    rec2_o = tail_a(p - 1, pq2_o)
                    hist[p - 1] = (nm2_o, rec2_o)
                hist[p] = (pq2, nm2)

            # drain: tail_a(3), tail_b(2), tail_b(3)
            pq2_l, nm2_l = hist.pop(NPAIR - 1)
            rec2_l = tail_a(NPAIR - 1, pq2_l)
            nm2_o, rec2_o = hist.pop(NPAIR - 2)
            tail_b(NPAIR - 2, nm2_o, rec2_o)
            tail_b(NPAIR - 1, nm2_l, rec2_l)

    nc.compile()
    return nc


_NC_CACHE = {}


def _get_nc():
    if "nc" not in _NC_CACHE:
        _NC_CACHE["nc"] = build()
    return _NC_CACHE["nc"]


def make_in_maps(x, Wq, bq, Wk, Wv, bv, Wo, bo):
    bf = ml_dtypes.bfloat16
    f8 = ml_dtypes.float8_e4m3
    WS = 1.0 / (XS * PS)  # weight pre-scale so that psum * PS = W @ x exactly
    x3 = np.asarray(x, np.float32)[..., 0]                      # (B, C, T)
    E = np.zeros((CG, 8), np.float32)
    for ch in range(CG):
        E[ch, ch // DH] = 1.0
    em = np.ascontiguousarray(E.reshape(NCH, 128, 8).transpose(1, 0, 2))
    ones = np.ones((128, P2), bf)
    in_maps = []
    for core in range(8):
        b, g = core // 2, core % 2
        sl = slice(CG * g, CG * (g + 1))
        in_maps.append({
            "x": np.clip(x3[b] * XS, -240, 240).astype(f8),
            "wq": np.clip(np.ascontiguousarray(np.asarray(Wq, np.float32)[sl, :].T) * WS, -240, 240).astype(f8),
            "wk": np.clip(np.ascontiguousarray(np.asarray(Wk, np.float32)[sl, :].T) * WS, -240, 240).astype(f8),
            "wv": np.clip(np.ascontiguousarray(np.asarray(Wv, np.float32)[sl, :].T) * WS, -240, 240).astype(f8),
            "wo": np.clip(np.ascontiguousarray(np.asarray(Wo, np.float32)[:, sl].T) * 8.0, -240, 240).astype(f8),
            "bq": np.ascontiguousarray(np.asarray(bq, np.float32)[sl].reshape(NCH, 128).T),
            "bv8": (np.asarray(bv, np.float32)[sl] * 8.0).reshape(1, CG).astype(bf),
            "emat": em.astype(bf),
            "fmat": np.ascontiguousarray(E.T / 8.0).astype(bf),
            "ones": ones,
        })
    return in_maps


def assemble(results, bo):
    out = np.empty((B, C, T, 1), np.float32)
    bo_f = np.asarray(bo, np.float32)[:, None]
    for b in range(B):
        p0 = np.asarray(results[2 * b]["out"], np.float32)
        p1 = np.asarray(results[2 * b + 1]["out"], np.float32)
        out[b, :, :, 0] = (p0 + p1) * (1.0 / 64.0) + bo_f
    return out


def kernel(x, Wq, bq, Wk, Wv, bv, Wo, bo):
    nc = _get_nc()
    in_maps = make_in_maps(x, Wq, bq, Wk, Wv, bv, Wo, bo)
    res = run_bass_kernel_spmd(nc, in_maps, core_ids=list(range(8)))
    return assemble(res.results, bo)


# revision 42
# speedup vs baseline: 1.1386x; 1.1386x over previous
"""Trainium2 Bass kernel for linear attention (silu+1 feature map, cumsum over T)
with dense 1024x1024 in/out projections.

Sharding: 8 cores = 4 batches x 2 head-groups (8 heads / 512 channels each).
Each core computes q/k/v projections for its 512 channels over the full
T=4096 of its batch, the linear-attention recurrence locally, and a partial
Wo projection (512 in-ch -> all 1024 out-ch). The host sums the two bf16
partials per batch, scales by 1/64 and adds bo. No cross-core traffic.

Engine assignment (vs the original version):
 - v-bias folded into the Wv matmul as a rank-1 fp8 DoubleRow ones-row update;
   phi_k*v (pk) reads the v PSUM directly (no ACT copy of v).
 - den: reciprocal reads the PSUM accumulator directly (eps dropped: den>=33),
   1/8 folded into the fm broadcast matrix (no ACT dens copy, no rec scale).
 - cumsums/phi_q/pq/nm run at FD=1024 (slab pairs) mid-stream to amortize
   per-op init; first and last pairs run their scan/tail stages at FD=512 to
   shorten pipeline ramp and drain.
 - pq/nm multiplies on the GPSIMD/Pool engine (TensorTensor is the only
   vector op the Pool engine supports on trn2 - scans/tensor_scalar are
   rejected by the ISA engine check).
 - output partials in bf16 (half the write traffic); input DMAs merged
   (1 per x slab, 1 per weight); out DMAs issued from SP a full pair late
   so the in-order SP DGE queue never blocks x loads.
"""

import numpy as np
import ml_dtypes

import concourse.bass as bass
import concourse.mybir as mybir
from concourse import bacc, tile
from concourse.bass_utils import run_bass_kernel_spmd

BF16 = mybir.dt.bfloat16
F32 = mybir.dt.float32
FP8 = mybir.dt.float8e4
DR = mybir.MatmulPerfMode.DoubleRow
XS = 0.125        # host scales x by XS, weights by 1/(XS*PS)
PS = 0.125        # ACT scale undoing the fp8 pre-scaling: psum*PS = true value
ADD = mybir.AluOpType.add
MULT = mybir.AluOpType.mult
BYPASS = mybir.AluOpType.bypass
SILU = mybir.ActivationFunctionType.Silu
COPY = mybir.ActivationFunctionType.Copy

B, C, T = 4, 1024, 4096
H, DH = 16, 64
CG = 512            # channels per head-group (per core)
S = 512             # projection slab (PSUM-bound)
P2 = 2 * S          # elementwise pair width
NPAIR = T // P2     # 4
NCH = CG // 128     # 4 chunks of 128 channels
KCH = C // 128      # 8 input-channel chunks
MO = C // 128       # 8 output-channel chunks


def build():
    """Build the per-core Bass program (identical on all 8 cores)."""
    nc = bacc.Bacc(target_bir_lowering=False)

    x_d = nc.declare_dram_parameter("x", [C, T], FP8, isOutput=False)
    wq_d = nc.declare_dram_parameter("wq", [C, CG], FP8, isOutput=False)
    wk_d = nc.declare_dram_parameter("wk", [C, CG], FP8, isOutput=False)
    wv_d = nc.declare_dram_parameter("wv", [C, CG], FP8, isOutput=False)
    wo_d = nc.declare_dram_parameter("wo", [CG, C], FP8, isOutput=False)
    bq_d = nc.declare_dram_parameter("bq", [128, NCH], F32, isOutput=False)
    bv_d = nc.declare_dram_parameter("bv8", [1, 2 * CG], FP8, isOutput=False)
    em_d = nc.declare_dram_parameter("emat", [128, NCH, 8], BF16, isOutput=False)
    fm_d = nc.declare_dram_parameter("fmat", [8, CG], BF16, isOutput=False)
    on_d = nc.declare_dram_parameter("ones", [128, P2], BF16, isOutput=False)
    oz_d = nc.declare_dram_parameter("onz", [1, 2 * S], FP8, isOutput=False)
    out_d = nc.declare_dram_parameter("out", [C, T], BF16, isOutput=True)

    with tile.TileContext(nc) as tc:
        from contextlib import ExitStack

        with ExitStack() as ctx:
            wpool = ctx.enter_context(tc.tile_pool(name="w", bufs=1))
            xpool = ctx.enter_context(tc.tile_pool(name="xp", bufs=3))
            ppool = ctx.enter_context(tc.tile_pool(name="proj", bufs=3, space="PSUM"))
            dpool = ctx.enter_context(tc.tile_pool(name="denp", bufs=1, space="PSUM"))
            bpool = ctx.enter_context(tc.tile_pool(name="bcast", bufs=2, space="PSUM"))
            opool = ctx.enter_context(tc.tile_pool(name="wops", bufs=2, space="PSUM"))
            apool = ctx.enter_context(tc.tile_pool(name="act", bufs=2))
            spool = ctx.enter_context(tc.tile_pool(name="state", bufs=2))
            outpool = ctx.enter_context(tc.tile_pool(name="outp", bufs=2))
            otpool = ctx.enter_context(tc.tile_pool(name="otp", bufs=3))

            wq_t = wk_t = wv_t = wo_t = None
            em_t = fm_t = bq_t = bv_t = ones_t = onz_t = None

            prev_ks = [None] * NCH   # (tile, last_col) per chunk
            prev_kvs = [None] * NCH

            def load_weights():
                nonlocal wq_t, wk_t, wv_t, wo_t, em_t, fm_t, bq_t, bv_t
                nonlocal ones_t, onz_t

                def ld(shape, dt, src, tag):
                    t = wpool.tile(shape, dt, tag=tag, name=tag)
                    nc.sync.dma_start(t[:], src)
                    return t

                wk_t = ld([128, KCH, CG], FP8,
                          wk_d.rearrange("(ko ki) m -> ki ko m", ko=KCH), "wk")
                ones_t = ld([128, P2], BF16, on_d[:, :], "ones")
                wq_t = ld([128, KCH, CG], FP8,
                          wq_d.rearrange("(ko ki) m -> ki ko m", ko=KCH), "wq")
                bq_t = ld([128, NCH], F32, bq_d[:, :], "bq")
                wv_t = ld([128, KCH, CG], FP8,
                          wv_d.rearrange("(ko ki) m -> ki ko m", ko=KCH), "wv")
                bv_t = ld([1, 2, CG], FP8,
                          bv_d.rearrange("p (ko m) -> p ko m", ko=2), "bv8")
                onz_t = ld([1, 2, S], FP8,
                           oz_d.rearrange("p (ko t) -> p ko t", ko=2), "onz")
                em_t = ld([128, NCH, 8], BF16, em_d[:, :, :], "em")
                fm_t = ld([8, CG], BF16, fm_d[:, :], "fm")
                wo_t = ld([128, NCH, C], FP8,
                          wo_d.rearrange("(ko ki) m -> ki ko m", ko=NCH), "wo")

            def stage_a(p, si, sq2, sk2, pk2, ramp_tiles=None, extras=None):
                """One 512-token slab: x DMA, k/q/v projections, silu, pk.

                ramp_tiles: when set (pipeline ramp), the per-chunk scan ops
                are interleaved right behind their producers so the DVE
                starts as soon as the first silu lands."""
                t0 = P2 * p + S * si
                hs = slice(S * si, S * si + S)
                x_t = xpool.tile([128, KCH, S], FP8, tag="x", name=f"x{p}_{si}")
                nc.sync.dma_start(
                    x_t[:], x_d[:, t0:t0 + S].rearrange("(ko ki) t -> ki ko t", ko=KCH))
                if wq_t is None:
                    load_weights()
                K2 = KCH // 2
                for c in range(NCH):
                    cs = slice(128 * c, 128 * (c + 1))
                    # k first: silu_k -> pk frees the v PSUM bank in time for
                    # the next chunk's v matmuls (no PE stall with 3 banks)
                    ps_k = ppool.tile([128, S], F32, tag="proj", name=f"psk{p}{si}_{c}")
                    for k in range(K2):
                        nc.tensor.matmul(ps_k[:], wk_t[:, 2 * k:2 * k + 2, cs],
                                         x_t[:, 2 * k:2 * k + 2, :],
                                         start=(k == 0), stop=(k == K2 - 1), perf_mode=DR)
                    nc.scalar.activation(sk2[c][:, hs], ps_k[:], SILU, scale=PS)
                    if ramp_tiles is not None:
                        _, _, _, _, ks2, _, _, _ = ramp_tiles
                        ik = 0.0 if prev_ks[c] is None else \
                            prev_ks[c][0][:, prev_ks[c][1]:prev_ks[c][1] + 1]
                        nc.vector.tensor_tensor_scan(
                            ks2[c][:, hs], sk2[c][:, hs], ones_t[:, :S],
                            initial=ik, op0=ADD, op1=ADD)
                        prev_ks[c] = (ks2[c], S * si + S - 1)
                    ps_q = ppool.tile([128, S], F32, tag="proj", name=f"psq{p}{si}_{c}")
                    for k in range(K2):
                        nc.tensor.matmul(ps_q[:], wq_t[:, 2 * k:2 * k + 2, cs],
                                         x_t[:, 2 * k:2 * k + 2, :],
                                         start=(k == 0), stop=(k == K2 - 1), perf_mode=DR)
                    nc.scalar.activation(sq2[c][:, hs], ps_q[:], SILU,
                                         bias=bq_t[:, c:c + 1], scale=PS)
                    ps_v = ppool.tile([128, S], F32, tag="proj", name=f"psv{p}{si}_{c}")
                    # rank-1 fp8-DR ones-row update adds 8*bv into the raw v PSUM
                    nc.tensor.matmul(ps_v[:], bv_t[:, :, cs], onz_t[:, :, :],
                                     start=True, stop=False, perf_mode=DR)
                    for k in range(K2):
                        nc.tensor.matmul(ps_v[:], wv_t[:, 2 * k:2 * k + 2, cs],
                                         x_t[:, 2 * k:2 * k + 2, :],
                                         start=False, stop=(k == K2 - 1), perf_mode=DR)
                    # pk = (silu_k + 1) * (8*v)   [raw 8x scale, fixed via fm]
                    nc.vector.scalar_tensor_tensor(
                        pk2[c][:, hs], sk2[c][:, hs], 1.0, ps_v[:], op0=ADD, op1=MULT)
                    if ramp_tiles is not None:
                        _, _, _, phq2, ks2, kvs2, pq2, nm2 = ramp_tiles
                        nc.vector.tensor_scalar_add(phq2[c][:, hs],
                                                    sq2[c][:, hs], 1.0)
                        ikv = 0.0 if prev_kvs[c] is None else \
                            prev_kvs[c][0][:, prev_kvs[c][1]:prev_kvs[c][1] + 1]
                        nc.vector.tensor_tensor_scan(
                            kvs2[c][:, hs], pk2[c][:, hs], ones_t[:, :S],
                            initial=ikv, op0=ADD, op1=BYPASS)
                        prev_kvs[c] = (kvs2[c], S * si + S - 1)
                        nc.gpsimd.tensor_mul(pq2[c][:, hs], phq2[c][:, hs],
                                             ks2[c][:, hs])
                        nc.gpsimd.tensor_mul(nm2[c][:, hs], phq2[c][:, hs],
                                             kvs2[c][:, hs])
                    if extras is not None:
                        extras(c, si)

            def scan_g(p, h0, nh, tiles, drain=False):
                """phi_q, cumsums, pq, nm on columns [h0*S, (h0+nh)*S)."""
                sq2, sk2, pk2, phq2, ks2, kvs2, pq2, nm2 = tiles
                sl = slice(h0 * S, (h0 + nh) * S)
                for c in range(NCH):
                    nc.vector.tensor_scalar_add(phq2[c][:, sl], sq2[c][:, sl], 1.0)
                    ik = 0.0 if prev_ks[c] is None else \
                        prev_ks[c][0][:, prev_ks[c][1]:prev_ks[c][1] + 1]
                    nc.vector.tensor_tensor_scan(ks2[c][:, sl], sk2[c][:, sl],
                                                 ones_t[:, :nh * S], initial=ik,
                                                 op0=ADD, op1=ADD)
                    ikv = 0.0 if prev_kvs[c] is None else \
                        prev_kvs[c][0][:, prev_kvs[c][1]:prev_kvs[c][1] + 1]
                    nc.vector.tensor_tensor_scan(kvs2[c][:, sl], pk2[c][:, sl],
                                                 ones_t[:, :nh * S], initial=ikv,
                                                 op0=ADD, op1=BYPASS)
                    prev_ks[c] = (ks2[c], (h0 + nh) * S - 1)
                    prev_kvs[c] = (kvs2[c], (h0 + nh) * S - 1)
                    eng = nc.vector if drain else nc.gpsimd
                    eng.tensor_mul(pq2[c][:, sl], phq2[c][:, sl], ks2[c][:, sl])
                    eng.tensor_mul(nm2[c][:, sl], phq2[c][:, sl], kvs2[c][:, sl])

            rec2_hist = {}

            def tail_a(p, hlist, pq2):
                """den -> reciprocal -> bf16 rec for the given halves of pair p."""
                if p not in rec2_hist:
                    rec2_hist[p] = apool.tile([8, P2], BF16, tag="rec2",
                                              name=f"rec2{p}")
                rec2 = rec2_hist[p]
                for h in hlist:
                    hs = slice(S * h, S * h + S)
                    den_ps = dpool.tile([8, S], F32, tag="den", name=f"den{p}_{h}")
                    for c in range(NCH):
                        nc.tensor.matmul(den_ps[:], em_t[:, c, :], pq2[c][:, hs],
                                         start=(c == 0), stop=(c == NCH - 1))
                    rec32 = apool.tile([8, S], F32, tag="rec32", name=f"rec32{p}_{h}")
                    nc.vector.reciprocal_approx_fast(rec32[:], den_ps[:])
                    nc.scalar.copy(rec2[:, hs], rec32[:])
                return rec2

            ot_hist = {}
            at_hist = {}

            def tail_b1_one(p, c, h, nm2, rec2):
                if p not in at_hist:
                    at_hist[p] = [outpool.tile([128, 2, P2], FP8, tag=f"at{kk}",
                                               name=f"at{p}_{kk}")
                                  for kk in range(NCH // 2)]
                at_l = at_hist[p]
                hs = slice(S * h, S * h + S)
                cs = slice(128 * c, 128 * (c + 1))
                rb = bpool.tile([128, S], F32, tag="rb", name=f"rb{p}_{c}{h}")
                nc.tensor.matmul(rb[:], fm_t[:, cs], rec2[:, hs],
                                 start=True, stop=True)
                nc.vector.tensor_mul(at_l[c // 2][:, c % 2, hs],
                                     nm2[c][:, hs], rb[:])

            def tail_b1(p, hlist, nm2, rec2):
                """broadcast -> attn (fp8)."""
                for h in hlist:
                    for c in range(NCH):
                        tail_b1_one(p, c, h, nm2, rec2)

            def tail_b2(p, hlist, drain=False):
                """Wo matmuls -> bf16 out copies."""
                at_l = at_hist[p]
                for h in hlist:
                    hs = slice(S * h, S * h + S)
                    for moo in range(MO // 2):
                        ot = otpool.tile([128, 2, S], BF16, tag=f"ot{moo}",
                                         name=f"ot{p}_{h}_{moo}")
                        for mo2 in range(2):
                            mo = 2 * moo + mo2
                            ms = slice(128 * mo, 128 * (mo + 1))
                            wo_ps = opool.tile([128, S], F32, tag="wo",
                                               name=f"wo{p}_{h}_{mo}")
                            for kk in range(NCH // 2):
                                nc.tensor.matmul(
                                    wo_ps[:], wo_t[:, 2 * kk:2 * kk + 2, ms],
                                    at_l[kk][:, :, hs],
                                    start=(kk == 0), stop=(kk == NCH // 2 - 1),
                                    perf_mode=DR)
                            if drain and mo2 == 1:
                                nc.vector.tensor_copy(ot[:, mo2, :], wo_ps[:])
                            else:
                                nc.scalar.copy(ot[:, mo2, :], wo_ps[:])
                        ot_hist[(p, h, moo)] = ot

            def out_dma(p):
                """Issue the 8 out DMAs for pair p from SP, a full pair after
                tail_b(p): every wait is satisfied at issue time, so the
                in-order SP DGE queue never blocks x loads."""
                for h in range(2):
                    tts = slice(P2 * p + S * h, P2 * p + S * h + S)
                    for moo in range(MO // 2):
                        ot = ot_hist.pop((p, h, moo))
                        nc.sync.dma_start(
                            out_d[256 * moo:256 * (moo + 1), tts].rearrange(
                                "(mo2 ki) t -> ki mo2 t", mo2=2),
                            ot[:])

            def alloc_pair(p):
                def mk(pool, tag):
                    return [pool.tile([128, P2], BF16, tag=f"{tag}{c}",
                                      name=f"{tag}{p}_{c}")
                            for c in range(NCH)]
                return (mk(apool, "sq"), mk(apool, "sk"), mk(apool, "pk"),
                        mk(apool, "phq"), mk(spool, "ks"), mk(spool, "kvs"),
                        mk(apool, "pq"), mk(apool, "nm"))

            hist = {}
            for p in range(NPAIR):
                if p >= 3:
                    out_dma(p - 3)
                tiles = alloc_pair(p)
                sq2, sk2, pk2 = tiles[0], tiles[1], tiles[2]
                pq2, nm2 = tiles[6], tiles[7]
                if p == 0:
                    # ramp: per-chunk interleaved scans so downstream starts early
                    stage_a(p, 0, sq2, sk2, pk2, ramp_tiles=tiles)
                    stage_a(p, 1, sq2, sk2, pk2, ramp_tiles=tiles)
                elif p < NPAIR - 1:
                    stage_a(p, 0, sq2, sk2, pk2, ramp_tiles=tiles)
                    stage_a(p, 1, sq2, sk2, pk2, ramp_tiles=tiles)
                    pq2_o, nm2_o = hist.pop(p - 1)
                    rec2_o = tail_a(p - 1, [0, 1], pq2_o)
                    hist[p - 1] = (nm2_o, rec2_o)
                    if p >= 2:
                        nm2_o, rec2_o = hist[p - 2]
                        tail_b1(p - 2, [0, 1], nm2_o, rec2_o)
                        tail_b2(p - 2, [0, 1])
                else:
                    # drain: 512-granular scans/tails for the last pair
                    stage_a(p, 0, sq2, sk2, pk2)
                    scan_g(p, 0, 1, tiles)
                    stage_a(p, 1, sq2, sk2, pk2)
                    nm2_o, rec2_o = hist[p - 2]
                    tail_b1(p - 2, [0, 1], nm2_o, rec2_o)
                    tail_b2(p - 2, [0, 1])
                    scan_g(p, 1, 1, tiles, drain=True)
                    pq2_o, nm2_o = hist.pop(p - 1)
                    rec2_o = tail_a(p - 1, [0, 1], pq2_o)
                    hist[p - 1] = (nm2_o, rec2_o)
                hist[p] = (pq2, nm2)

            # drain
            out_dma(1)
            nm2_o, rec2_o = hist.pop(NPAIR - 2)
            tail_b1(NPAIR - 2, [0, 1], nm2_o, rec2_o)
            tail_b2(NPAIR - 2, [0, 1], drain=True)
            pq2_l, nm2_l = hist.pop(NPAIR - 1)
            rec2_l = tail_a(NPAIR - 1, [0], pq2_l)
            tail_b1(NPAIR - 1, [0], nm2_l, rec2_l)
            tail_b2(NPAIR - 1, [0], drain=True)
            tail_a(NPAIR - 1, [1], pq2_l)
            tail_b1(NPAIR - 1, [1], nm2_l, rec2_l)
            tail_b2(NPAIR - 1, [1], drain=True)
            out_dma(NPAIR - 2)
            out_dma(NPAIR - 1)

    nc.compile()
    return nc


_NC_CACHE = {}


def _get_nc():
    if "nc" not in _NC_CACHE:
        _NC_CACHE["nc"] = build()
    return _NC_CACHE["nc"]


def make_in_maps(x, Wq, bq, Wk, Wv, bv, Wo, bo):
    bf = ml_dtypes.bfloat16
    f8 = ml_dtypes.float8_e4m3
    WS = 1.0 / (XS * PS)  # weight pre-scale so that psum * PS = W @ x exactly
    x3 = np.asarray(x, np.float32)[..., 0]                      # (B, C, T)
    E = np.zeros((CG, 8), np.float32)
    for ch in range(CG):
        E[ch, ch // DH] = 1.0
    em = np.ascontiguousarray(E.reshape(NCH, 128, 8).transpose(1, 0, 2))
    ones = np.ones((128, P2), bf)
    in_maps = []
    for core in range(8):
        b, g = core // 2, core % 2
        sl = slice(CG * g, CG * (g + 1))
        in_maps.append({
            "x": np.clip(x3[b] * XS, -240, 240).astype(f8),
            "wq": np.clip(np.ascontiguousarray(np.asarray(Wq, np.float32)[sl, :].T) * WS, -240, 240).astype(f8),
            "wk": np.clip(np.ascontiguousarray(np.asarray(Wk, np.float32)[sl, :].T) * WS, -240, 240).astype(f8),
            "wv": np.clip(np.ascontiguousarray(np.asarray(Wv, np.float32)[sl, :].T) * WS, -240, 240).astype(f8),
            "wo": np.clip(np.ascontiguousarray(np.asarray(Wo, np.float32)[:, sl].T) * 8.0, -240, 240).astype(f8),
            "bq": np.ascontiguousarray(np.asarray(bq, np.float32)[sl].reshape(NCH, 128).T),
            "bv8": np.clip(np.stack([np.asarray(bv, np.float32)[sl] * 8.0,
                                     np.zeros(CG, np.float32)]), -240, 240).reshape(1, 2 * CG).astype(f8),
            "emat": em.astype(bf),
            "fmat": np.ascontiguousarray(E.T / 8.0).astype(bf),
            "ones": ones,
            "onz": np.stack([np.ones(S, np.float32),
                             np.zeros(S, np.float32)]).reshape(1, 2 * S).astype(f8),
        })
    return in_maps


def assemble(results, bo):
    out = np.empty((B, C, T, 1), np.float32)
    bo_f = np.asarray(bo, np.float32)[:, None]
    for b in range(B):
        p0 = np.asarray(results[2 * b]["out"], np.float32)
        p1 = np.asarray(results[2 * b + 1]["out"], np.float32)
        out[b, :, :, 0] = (p0 + p1) * (1.0 / 64.0) + bo_f
    return out


def kernel(x, Wq, bq, Wk, Wv, bv, Wo, bo):
    nc = _get_nc()
    in_maps = make_in_maps(x, Wq, bq, Wk, Wv, bv, Wo, bo)
    res = run_bass_kernel_spmd(nc, in_maps, core_ids=list(range(8)))
    return assemble(res.results, bo)


# revision 53
# speedup vs baseline: 1.1619x; 1.0205x over previous
"""Trainium2 Bass kernel for linear attention (silu+1 feature map, cumsum over T)
with dense 1024x1024 in/out projections.

Sharding: 8 cores = 4 batches x 2 head-groups (8 heads / 512 channels each).
Each core computes q/k/v projections for its 512 channels over the full
T=4096 of its batch, the linear-attention recurrence locally, and a partial
Wo projection (512 in-ch -> all 1024 out-ch). The host sums the two bf16
partials per batch, scales by 1/64 and adds bo. No cross-core traffic.

Engine assignment (vs the original version):
 - v-bias folded into the Wv matmul as a rank-1 fp8 DoubleRow ones-row update;
   phi_k*v (pk) reads the v PSUM directly (no ACT copy of v).
 - den: reciprocal reads the PSUM accumulator directly (eps dropped: den>=33),
   1/8 folded into the fm broadcast matrix (no ACT dens copy, no rec scale).
 - all scan-stage ops run at FD=512, interleaved per-chunk directly behind
   their producers inside stage_a (ks right after silu_k, kvs/pq/nm right
   after pk): the fine granularity unblocks the Pool->den->recip chain a
   slab earlier than pair-level blocks and sustains ~77% DVE occupancy.
 - phi_q (+1) on ACT right behind silu_q; pq/nm multiplies on the
   GPSIMD/Pool engine (TensorTensor is the only vector op the Pool engine
   supports on trn2 - scans/tensor_scalar are rejected by the ISA engine
   check); the last pair computes pq/nm on DVE to shorten the drain.
 - output partials in bf16 (half the write traffic); input DMAs merged
   (1 per x slab, 1 per weight); out DMAs issued from SP a full pair late
   so the in-order SP DGE queue never blocks x loads.
"""

import numpy as np
import ml_dtypes

import concourse.bass as bass
import concourse.mybir as mybir
from concourse import bacc, tile
from concourse.bass_utils import run_bass_kernel_spmd

BF16 = mybir.dt.bfloat16
F32 = mybir.dt.float32
FP8 = mybir.dt.float8e4
DR = mybir.MatmulPerfMode.DoubleRow
XS = 0.125        # host scales x by XS, weights by 1/(XS*PS)
PS = 0.125        # ACT scale undoing the fp8 pre-scaling: psum*PS = true value
ADD = mybir.AluOpType.add
MULT = mybir.AluOpType.mult
BYPASS = mybir.AluOpType.bypass
SILU = mybir.ActivationFunctionType.Silu
COPY = mybir.ActivationFunctionType.Copy

B, C, T = 4, 1024, 4096
H, DH = 16, 64
CG = 512            # channels per head-group (per core)
S = 512             # projection slab (PSUM-bound)
P2 = 2 * S          # elementwise pair width
NPAIR = T // P2     # 4
NCH = CG // 128     # 4 chunks of 128 channels
KCH = C // 128      # 8 input-channel chunks
MO = C // 128       # 8 output-channel chunks


def build():
    """Build the per-core Bass program (identical on all 8 cores)."""
    nc = bacc.Bacc(target_bir_lowering=False)

    x_d = nc.declare_dram_parameter("x", [C, T], FP8, isOutput=False)
    wq_d = nc.declare_dram_parameter("wq", [C, CG], FP8, isOutput=False)
    wk_d = nc.declare_dram_parameter("wk", [C, CG], FP8, isOutput=False)
    wv_d = nc.declare_dram_parameter("wv", [C, CG], FP8, isOutput=False)
    wo_d = nc.declare_dram_parameter("wo", [CG, C], FP8, isOutput=False)
    bq_d = nc.declare_dram_parameter("bq", [128, NCH], F32, isOutput=False)
    bv_d = nc.declare_dram_parameter("bv8", [1, 2 * CG], FP8, isOutput=False)
    em_d = nc.declare_dram_parameter("emat", [128, NCH, 8], BF16, isOutput=False)
    fm_d = nc.declare_dram_parameter("fmat", [8, CG], BF16, isOutput=False)
    on_d = nc.declare_dram_parameter("ones", [128, P2], BF16, isOutput=False)
    oz_d = nc.declare_dram_parameter("onz", [1, 2 * S], FP8, isOutput=False)
    out_d = nc.declare_dram_parameter("out", [C, T], BF16, isOutput=True)

    with tile.TileContext(nc) as tc:
        from contextlib import ExitStack

        with ExitStack() as ctx:
            wpool = ctx.enter_context(tc.tile_pool(name="w", bufs=1))
            xpool = ctx.enter_context(tc.tile_pool(name="xp", bufs=3))
            ppool = ctx.enter_context(tc.tile_pool(name="proj", bufs=3, space="PSUM"))
            dpool = ctx.enter_context(tc.tile_pool(name="denp", bufs=1, space="PSUM"))
            bpool = ctx.enter_context(tc.tile_pool(name="bcast", bufs=2, space="PSUM"))
            opool = ctx.enter_context(tc.tile_pool(name="wops", bufs=2, space="PSUM"))
            apool = ctx.enter_context(tc.tile_pool(name="act", bufs=2))
            spool = ctx.enter_context(tc.tile_pool(name="state", bufs=2))
            outpool = ctx.enter_context(tc.tile_pool(name="outp", bufs=2))
            otpool = ctx.enter_context(tc.tile_pool(name="otp", bufs=3))

            wq_t = wk_t = wv_t = wo_t = None
            em_t = fm_t = bq_t = bv_t = ones_t = onz_t = None

            prev_ks = [None] * NCH   # (tile, last_col) per chunk
            prev_kvs = [None] * NCH

            def load_weights():
                nonlocal wq_t, wk_t, wv_t, wo_t, em_t, fm_t, bq_t, bv_t
                nonlocal ones_t, onz_t

                def ld(shape, dt, src, tag):
                    t = wpool.tile(shape, dt, tag=tag, name=tag)
                    nc.sync.dma_start(t[:], src)
                    return t

                wk_t = ld([128, KCH, CG], FP8,
                          wk_d.rearrange("(ko ki) m -> ki ko m", ko=KCH), "wk")
                ones_t = ld([128, P2], BF16, on_d[:, :], "ones")
                wq_t = ld([128, KCH, CG], FP8,
                          wq_d.rearrange("(ko ki) m -> ki ko m", ko=KCH), "wq")
                bq_t = ld([128, NCH], F32, bq_d[:, :], "bq")
                wv_t = ld([128, KCH, CG], FP8,
                          wv_d.rearrange("(ko ki) m -> ki ko m", ko=KCH), "wv")
                bv_t = ld([1, 2, CG], FP8,
                          bv_d.rearrange("p (ko m) -> p ko m", ko=2), "bv8")
                onz_t = ld([1, 2, S], FP8,
                           oz_d.rearrange("p (ko t) -> p ko t", ko=2), "onz")
                em_t = ld([128, NCH, 8], BF16, em_d[:, :, :], "em")
                fm_t = ld([8, CG], BF16, fm_d[:, :], "fm")
                wo_t = ld([128, NCH, C], FP8,
                          wo_d.rearrange("(ko ki) m -> ki ko m", ko=NCH), "wo")

            def stage_a(p, si, sq2, sk2, pk2, ramp_tiles=None, extras=None):
                """One 512-token slab: x DMA, k/q/v projections, silu, pk.

                ramp_tiles: when set (pipeline ramp), the per-chunk scan ops
                are interleaved right behind their producers so the DVE
                starts as soon as the first silu lands."""
                t0 = P2 * p + S * si
                hs = slice(S * si, S * si + S)
                x_t = xpool.tile([128, KCH, S], FP8, tag="x", name=f"x{p}_{si}")
                nc.sync.dma_start(
                    x_t[:], x_d[:, t0:t0 + S].rearrange("(ko ki) t -> ki ko t", ko=KCH))
                if wq_t is None:
                    load_weights()
                K2 = KCH // 2
                for c in range(NCH):
                    cs = slice(128 * c, 128 * (c + 1))
                    # k first: silu_k -> pk frees the v PSUM bank in time for
                    # the next chunk's v matmuls (no PE stall with 3 banks)
                    ps_k = ppool.tile([128, S], F32, tag="proj", name=f"psk{p}{si}_{c}")
                    for k in range(K2):
                        nc.tensor.matmul(ps_k[:], wk_t[:, 2 * k:2 * k + 2, cs],
                                         x_t[:, 2 * k:2 * k + 2, :],
                                         start=(k == 0), stop=(k == K2 - 1), perf_mode=DR)
                    nc.scalar.activation(sk2[c][:, hs], ps_k[:], SILU, scale=PS)
                    if ramp_tiles is not None:
                        _, _, _, _, ks2, _, _, _ = ramp_tiles
                        ik = 0.0 if prev_ks[c] is None else \
                            prev_ks[c][0][:, prev_ks[c][1]:prev_ks[c][1] + 1]
                        nc.vector.tensor_tensor_scan(
                            ks2[c][:, hs], sk2[c][:, hs], ones_t[:, :S],
                            initial=ik, op0=ADD, op1=ADD)
                        prev_ks[c] = (ks2[c], S * si + S - 1)
                    ps_q = ppool.tile([128, S], F32, tag="proj", name=f"psq{p}{si}_{c}")
                    for k in range(K2):
                        nc.tensor.matmul(ps_q[:], wq_t[:, 2 * k:2 * k + 2, cs],
                                         x_t[:, 2 * k:2 * k + 2, :],
                                         start=(k == 0), stop=(k == K2 - 1), perf_mode=DR)
                    nc.scalar.activation(sq2[c][:, hs], ps_q[:], SILU,
                                         bias=bq_t[:, c:c + 1], scale=PS)
                    if ramp_tiles is not None:
                        nc.scalar.activation(ramp_tiles[3][c][:, hs],
                                             sq2[c][:, hs], COPY, bias=1.0)
                    ps_v = ppool.tile([128, S], F32, tag="proj", name=f"psv{p}{si}_{c}")
                    # rank-1 fp8-DR ones-row update adds 8*bv into the raw v PSUM
                    nc.tensor.matmul(ps_v[:], bv_t[:, :, cs], onz_t[:, :, :],
                                     start=True, stop=False, perf_mode=DR)
                    for k in range(K2):
                        nc.tensor.matmul(ps_v[:], wv_t[:, 2 * k:2 * k + 2, cs],
                                         x_t[:, 2 * k:2 * k + 2, :],
                                         start=False, stop=(k == K2 - 1), perf_mode=DR)
                    # pk = (silu_k + 1) * (8*v)   [raw 8x scale, fixed via fm]
                    nc.vector.scalar_tensor_tensor(
                        pk2[c][:, hs], sk2[c][:, hs], 1.0, ps_v[:], op0=ADD, op1=MULT)
                    if ramp_tiles is not None:
                        _, _, _, phq2, ks2, kvs2, pq2, nm2 = ramp_tiles
                        ikv = 0.0 if prev_kvs[c] is None else \
                            prev_kvs[c][0][:, prev_kvs[c][1]:prev_kvs[c][1] + 1]
                        nc.vector.tensor_tensor_scan(
                            kvs2[c][:, hs], pk2[c][:, hs], ones_t[:, :S],
                            initial=ikv, op0=ADD, op1=BYPASS)
                        prev_kvs[c] = (kvs2[c], S * si + S - 1)
                        nc.gpsimd.tensor_mul(pq2[c][:, hs], phq2[c][:, hs],
                                             ks2[c][:, hs])
                        nc.gpsimd.tensor_mul(nm2[c][:, hs], phq2[c][:, hs],
                                             kvs2[c][:, hs])
                    if extras is not None:
                        extras(c, si)

            def scan_g(p, h0, nh, tiles, drain=False):
                """phi_q, cumsums, pq, nm on columns [h0*S, (h0+nh)*S)."""
                sq2, sk2, pk2, phq2, ks2, kvs2, pq2, nm2 = tiles
                sl = slice(h0 * S, (h0 + nh) * S)
                for c in range(NCH):
                    nc.vector.tensor_scalar_add(phq2[c][:, sl], sq2[c][:, sl], 1.0)
                    ik = 0.0 if prev_ks[c] is None else \
                        prev_ks[c][0][:, prev_ks[c][1]:prev_ks[c][1] + 1]
                    nc.vector.tensor_tensor_scan(ks2[c][:, sl], sk2[c][:, sl],
                                                 ones_t[:, :nh * S], initial=ik,
                                                 op0=ADD, op1=ADD)
                    ikv = 0.0 if prev_kvs[c] is None else \
                        prev_kvs[c][0][:, prev_kvs[c][1]:prev_kvs[c][1] + 1]
                    nc.vector.tensor_tensor_scan(kvs2[c][:, sl], pk2[c][:, sl],
                                                 ones_t[:, :nh * S], initial=ikv,
                                                 op0=ADD, op1=BYPASS)
                    prev_ks[c] = (ks2[c], (h0 + nh) * S - 1)
                    prev_kvs[c] = (kvs2[c], (h0 + nh) * S - 1)
                    eng = nc.vector if drain else nc.gpsimd
                    eng.tensor_mul(pq2[c][:, sl], phq2[c][:, sl], ks2[c][:, sl])
                    eng.tensor_mul(nm2[c][:, sl], phq2[c][:, sl], kvs2[c][:, sl])

            rec2_hist = {}

            def tail_a(p, hlist, pq2):
                """den -> reciprocal -> bf16 rec for the given halves of pair p."""
                if p not in rec2_hist:
                    rec2_hist[p] = apool.tile([8, P2], BF16, tag="rec2",
                                              name=f"rec2{p}")
                rec2 = rec2_hist[p]
                for h in hlist:
                    hs = slice(S * h, S * h + S)
                    den_ps = dpool.tile([8, S], F32, tag="den", name=f"den{p}_{h}")
                    for c in range(NCH):
                        nc.tensor.matmul(den_ps[:], em_t[:, c, :], pq2[c][:, hs],
                                         start=(c == 0), stop=(c == NCH - 1))
                    rec32 = apool.tile([8, S], F32, tag="rec32", name=f"rec32{p}_{h}")
                    nc.vector.reciprocal_approx_fast(rec32[:], den_ps[:])
                    nc.scalar.copy(rec2[:, hs], rec32[:])
                return rec2

            ot_hist = {}
            at_hist = {}

            def tail_b1_one(p, c, h, nm2, rec2):
                if p not in at_hist:
                    at_hist[p] = [outpool.tile([128, 2, P2], FP8, tag=f"at{kk}",
                                               name=f"at{p}_{kk}")
                                  for kk in range(NCH // 2)]
                at_l = at_hist[p]
                hs = slice(S * h, S * h + S)
                cs = slice(128 * c, 128 * (c + 1))
                rb = bpool.tile([128, S], F32, tag="rb", name=f"rb{p}_{c}{h}")
                nc.tensor.matmul(rb[:], fm_t[:, cs], rec2[:, hs],
                                 start=True, stop=True)
                nc.vector.tensor_mul(at_l[c // 2][:, c % 2, hs],
                                     nm2[c][:, hs], rb[:])

            def tail_b1(p, hlist, nm2, rec2):
                """broadcast -> attn (fp8)."""
                for h in hlist:
                    for c in range(NCH):
                        tail_b1_one(p, c, h, nm2, rec2)

            def tail_b2(p, hlist, drain=False):
                """Wo matmuls -> bf16 out copies."""
                at_l = at_hist[p]
                for h in hlist:
                    hs = slice(S * h, S * h + S)
                    for moo in range(MO // 2):
                        ot = otpool.tile([128, 2, S], BF16, tag=f"ot{moo}",
                                         name=f"ot{p}_{h}_{moo}")
                        for mo2 in range(2):
                            mo = 2 * moo + mo2
                            ms = slice(128 * mo, 128 * (mo + 1))
                            wo_ps = opool.tile([128, S], F32, tag="wo",
                                               name=f"wo{p}_{h}_{mo}")
                            for kk in range(NCH // 2):
                                nc.tensor.matmul(
                                    wo_ps[:], wo_t[:, 2 * kk:2 * kk + 2, ms],
                                    at_l[kk][:, :, hs],
                                    start=(kk == 0), stop=(kk == NCH // 2 - 1),
                                    perf_mode=DR)
                            if drain and mo2 == 1 and moo % 2 == 0:
                                nc.vector.tensor_copy(ot[:, mo2, :], wo_ps[:])
                            else:
                                nc.scalar.copy(ot[:, mo2, :], wo_ps[:])
                        ot_hist[(p, h, moo)] = ot

            def out_dma(p):
                """Issue the 8 out DMAs for pair p from SP, a full pair after
                tail_b(p): every wait is satisfied at issue time, so the
                in-order SP DGE queue never blocks x loads."""
                for h in range(2):
                    tts = slice(P2 * p + S * h, P2 * p + S * h + S)
                    for moo in range(MO // 2):
                        ot = ot_hist.pop((p, h, moo))
                        nc.sync.dma_start(
                            out_d[256 * moo:256 * (moo + 1), tts].rearrange(
                                "(mo2 ki) t -> ki mo2 t", mo2=2),
                            ot[:])

            def alloc_pair(p):
                def mk(pool, tag):
                    return [pool.tile([128, P2], BF16, tag=f"{tag}{c}",
                                      name=f"{tag}{p}_{c}")
                            for c in range(NCH)]
                return (mk(apool, "sq"), mk(apool, "sk"), mk(apool, "pk"),
                        mk(apool, "phq"), mk(spool, "ks"), mk(spool, "kvs"),
                        mk(apool, "pq"), mk(apool, "nm"))

            hist = {}
            for p in range(NPAIR):
                if p >= 3:
                    out_dma(p - 3)
                tiles = alloc_pair(p)
                sq2, sk2, pk2 = tiles[0], tiles[1], tiles[2]
                pq2, nm2 = tiles[6], tiles[7]
                if p == 0:
                    # ramp: per-chunk interleaved scans so downstream starts early
                    stage_a(p, 0, sq2, sk2, pk2, ramp_tiles=tiles)
                    stage_a(p, 1, sq2, sk2, pk2, ramp_tiles=tiles)
                elif p < NPAIR - 1:
                    stage_a(p, 0, sq2, sk2, pk2, ramp_tiles=tiles)
                    stage_a(p, 1, sq2, sk2, pk2, ramp_tiles=tiles)
                    pq2_o, nm2_o = hist.pop(p - 1)
                    rec2_o = tail_a(p - 1, [0, 1], pq2_o)
                    hist[p - 1] = (nm2_o, rec2_o)
                    if p >= 2:
                        nm2_o, rec2_o = hist[p - 2]
                        tail_b1(p - 2, [0, 1], nm2_o, rec2_o)
                        tail_b2(p - 2, [0, 1])
                else:
                    # drain: 512-granular scans/tails for the last pair
                    stage_a(p, 0, sq2, sk2, pk2)
                    scan_g(p, 0, 1, tiles)
                    stage_a(p, 1, sq2, sk2, pk2)
                    nm2_o, rec2_o = hist[p - 2]
                    tail_b1(p - 2, [0, 1], nm2_o, rec2_o)
                    tail_b2(p - 2, [0, 1])
                    scan_g(p, 1, 1, tiles, drain=True)
                    pq2_o, nm2_o = hist.pop(p - 1)
                    rec2_o = tail_a(p - 1, [0, 1], pq2_o)
                    hist[p - 1] = (nm2_o, rec2_o)
                hist[p] = (pq2, nm2)

            # drain
            out_dma(1)
            nm2_o, rec2_o = hist.pop(NPAIR - 2)
            tail_b1(NPAIR - 2, [0, 1], nm2_o, rec2_o)
            tail_b2(NPAIR - 2, [0, 1], drain=True)
            pq2_l, nm2_l = hist.pop(NPAIR - 1)
            rec2_l = tail_a(NPAIR - 1, [0], pq2_l)
            tail_b1(NPAIR - 1, [0], nm2_l, rec2_l)
            tail_b2(NPAIR - 1, [0], drain=True)
            tail_a(NPAIR - 1, [1], pq2_l)
            tail_b1(NPAIR - 1, [1], nm2_l, rec2_l)
            tail_b2(NPAIR - 1, [1], drain=True)
            out_dma(NPAIR - 2)
            out_dma(NPAIR - 1)

    nc.compile()
    return nc


_NC_CACHE = {}


def _get_nc():
    if "nc" not in _NC_CACHE:
        _NC_CACHE["nc"] = build()
    return _NC_CACHE["nc"]


def make_in_maps(x, Wq, bq, Wk, Wv, bv, Wo, bo):
    bf = ml_dtypes.bfloat16
    f8 = ml_dtypes.float8_e4m3
    WS = 1.0 / (XS * PS)  # weight pre-scale so that psum * PS = W @ x exactly
    x3 = np.asarray(x, np.float32)[..., 0]                      # (B, C, T)
    E = np.zeros((CG, 8), np.float32)
    for ch in range(CG):
        E[ch, ch // DH] = 1.0
    em = np.ascontiguousarray(E.reshape(NCH, 128, 8).transpose(1, 0, 2))
    ones = np.ones((128, P2), bf)
    in_maps = []
    for core in range(8):
        b, g = core // 2, core % 2
        sl = slice(CG * g, CG * (g + 1))
        in_maps.append({
            "x": np.clip(x3[b] * XS, -240, 240).astype(f8),
            "wq": np.clip(np.ascontiguousarray(np.asarray(Wq, np.float32)[sl, :].T) * WS, -240, 240).astype(f8),
            "wk": np.clip(np.ascontiguousarray(np.asarray(Wk, np.float32)[sl, :].T) * WS, -240, 240).astype(f8),
            "wv": np.clip(np.ascontiguousarray(np.asarray(Wv, np.float32)[sl, :].T) * WS, -240, 240).astype(f8),
            "wo": np.clip(np.ascontiguousarray(np.asarray(Wo, np.float32)[:, sl].T) * 8.0, -240, 240).astype(f8),
            "bq": np.ascontiguousarray(np.asarray(bq, np.float32)[sl].reshape(NCH, 128).T),
            "bv8": np.clip(np.stack([np.asarray(bv, np.float32)[sl] * 8.0,
                                     np.zeros(CG, np.float32)]), -240, 240).reshape(1, 2 * CG).astype(f8),
            "emat": em.astype(bf),
            "fmat": np.ascontiguousarray(E.T / 8.0).astype(bf),
            "ones": ones,
            "onz": np.stack([np.ones(S, np.float32),
                             np.zeros(S, np.float32)]).reshape(1, 2 * S).astype(f8),
        })
    return in_maps


def assemble(results, bo):
    out = np.empty((B, C, T, 1), np.float32)
    bo_f = np.asarray(bo, np.float32)[:, None]
    for b in range(B):
        p0 = np.asarray(results[2 * b]["out"], np.float32)
        p1 = np.asarray(results[2 * b + 1]["out"], np.float32)
        out[b, :, :, 0] = (p0 + p1) * (1.0 / 64.0) + bo_f
    return out


def kernel(x, Wq, bq, Wk, Wv, bv, Wo, bo):
    nc = _get_nc()
    in_maps = make_in_maps(x, Wq, bq, Wk, Wv, bv, Wo, bo)
    res = run_bass_kernel_spmd(nc, in_maps, core_ids=list(range(8)))
    return assemble(res.results, bo)


# revision 68
# speedup vs baseline: 1.1727x; 1.0092x over previous
"""Trainium2 Bass kernel for linear attention (silu+1 feature map, cumsum over T)
with dense 1024x1024 in/out projections.

Sharding: 8 cores = 4 batches x 2 head-groups (8 heads / 512 channels each).
Each core computes q/k/v projections for its 512 channels over the full
T=4096 of its batch, the linear-attention recurrence locally, and a partial
Wo projection (512 in-ch -> all 1024 out-ch). The host sums the two bf16
partials per batch, scales by 1/64 and adds bo. No cross-core traffic.

Engine assignment (vs the original version):
 - v-bias folded into the Wv matmul as a rank-1 fp8 DoubleRow ones-row update;
   phi_k*v (pk) reads the v PSUM directly (no ACT copy of v).
 - den: reciprocal reads the PSUM accumulator directly (eps dropped: den>=33),
   1/8 folded into the fm broadcast matrix (no ACT dens copy, no rec scale).
 - all scan-stage ops run at FD=512, interleaved per-chunk directly behind
   their producers inside stage_a (ks right after silu_k, kvs/pq/nm right
   after pk): the fine granularity unblocks the Pool->den->recip chain a
   slab earlier than pair-level blocks and sustains ~77% DVE occupancy.
 - phi_q (+1) on ACT right behind silu_q; pq/nm multiplies on the
   GPSIMD/Pool engine (TensorTensor is the only vector op the Pool engine
   supports on trn2 - scans/tensor_scalar are rejected by the ISA engine
   check); the last pair computes pq/nm on DVE to shorten the drain.
 - output partials in bf16 (half the write traffic); input DMAs merged
   (1 per x slab, 1 per weight); out DMAs issued from SP a full pair late
   so the in-order SP DGE queue never blocks x loads.
"""

import numpy as np
import ml_dtypes

import concourse.bass as bass
import concourse.mybir as mybir
from concourse import bacc, tile
from concourse.bass_utils import run_bass_kernel_spmd

BF16 = mybir.dt.bfloat16
F32 = mybir.dt.float32
FP8 = mybir.dt.float8e4
DR = mybir.MatmulPerfMode.DoubleRow
XS = 0.125        # host scales x by XS, weights by 1/(XS*PS)
PS = 0.125        # ACT scale undoing the fp8 pre-scaling: psum*PS = true value
ADD = mybir.AluOpType.add
MULT = mybir.AluOpType.mult
BYPASS = mybir.AluOpType.bypass
SILU = mybir.ActivationFunctionType.Silu
COPY = mybir.ActivationFunctionType.Copy

B, C, T = 4, 1024, 4096
H, DH = 16, 64
CG = 512            # channels per head-group (per core)
S = 512             # projection slab (PSUM-bound)
P2 = 2 * S          # elementwise pair width
NPAIR = T // P2     # 4
NCH = CG // 128     # 4 chunks of 128 channels
KCH = C // 128      # 8 input-channel chunks
MO = C // 128       # 8 output-channel chunks


def build():
    """Build the per-core Bass program (identical on all 8 cores)."""
    nc = bacc.Bacc(target_bir_lowering=False)

    x_d = nc.declare_dram_parameter("x", [128, (T // S) * KCH * S], FP8, isOutput=False)
    wq_d = nc.declare_dram_parameter("wq", [128, KCH * CG], FP8, isOutput=False)
    wk_d = nc.declare_dram_parameter("wk", [128, KCH * CG], FP8, isOutput=False)
    wv_d = nc.declare_dram_parameter("wv", [128, KCH * CG], FP8, isOutput=False)
    wo_d = nc.declare_dram_parameter("wo", [128, NCH * C], FP8, isOutput=False)
    bq_d = nc.declare_dram_parameter("bq", [128, NCH], F32, isOutput=False)
    bv_d = nc.declare_dram_parameter("bv8", [1, 2 * CG], FP8, isOutput=False)
    em_d = nc.declare_dram_parameter("emat", [128, NCH, 8], BF16, isOutput=False)
    fm_d = nc.declare_dram_parameter("fmat", [8, CG], BF16, isOutput=False)
    on_d = nc.declare_dram_parameter("ones", [128, P2], BF16, isOutput=False)
    oz_d = nc.declare_dram_parameter("onz", [1, 2 * S], FP8, isOutput=False)
    out_d = nc.declare_dram_parameter("out", [128, (T // S) * MO * S], BF16, isOutput=True)

    with tile.TileContext(nc) as tc:
        from contextlib import ExitStack

        with ExitStack() as ctx:
            wpool = ctx.enter_context(tc.tile_pool(name="w", bufs=1))
            xpool = ctx.enter_context(tc.tile_pool(name="xp", bufs=3))
            ppool = ctx.enter_context(tc.tile_pool(name="proj", bufs=3, space="PSUM"))
            dpool = ctx.enter_context(tc.tile_pool(name="denp", bufs=1, space="PSUM"))
            bpool = ctx.enter_context(tc.tile_pool(name="bcast", bufs=2, space="PSUM"))
            opool = ctx.enter_context(tc.tile_pool(name="wops", bufs=2, space="PSUM"))
            apool = ctx.enter_context(tc.tile_pool(name="act", bufs=2))
            spool = ctx.enter_context(tc.tile_pool(name="state", bufs=2))
            outpool = ctx.enter_context(tc.tile_pool(name="outp", bufs=2))
            otpool = ctx.enter_context(tc.tile_pool(name="otp", bufs=3))

            wq_t = wk_t = wv_t = wo_t = None
            em_t = fm_t = bq_t = bv_t = ones_t = onz_t = None

            prev_ks = [None] * NCH   # (tile, last_col) per chunk
            prev_kvs = [None] * NCH

            def load_wk():
                nonlocal wk_t

                def ld(shape, dt, src, tag):
                    t = wpool.tile(shape, dt, tag=tag, name=tag)
                    nc.sync.dma_start(t[:], src)
                    return t

                wk_t = ld([128, KCH, CG], FP8,
                          wk_d.rearrange("p (ko m) -> p ko m", ko=KCH), "wk")

            def load_weights():
                nonlocal wq_t, wv_t, wo_t, em_t, fm_t, bq_t, bv_t
                nonlocal ones_t, onz_t

                def ld(shape, dt, src, tag):
                    t = wpool.tile(shape, dt, tag=tag, name=tag)
                    nc.sync.dma_start(t[:], src)
                    return t

                ones_t = ld([128, P2], BF16, on_d[:, :], "ones")
                wq_t = ld([128, KCH, CG], FP8,
                          wq_d.rearrange("p (ko m) -> p ko m", ko=KCH), "wq")
                bq_t = ld([128, NCH], F32, bq_d[:, :], "bq")
                wv_t = ld([128, KCH, CG], FP8,
                          wv_d.rearrange("p (ko m) -> p ko m", ko=KCH), "wv")
                bv_t = ld([1, 2, CG], FP8,
                          bv_d.rearrange("p (ko m) -> p ko m", ko=2), "bv8")
                onz_t = ld([1, 2, S], FP8,
                           oz_d.rearrange("p (ko t) -> p ko t", ko=2), "onz")
                em_t = ld([128, NCH, 8], BF16, em_d[:, :, :], "em")
                fm_t = ld([8, CG], BF16, fm_d[:, :], "fm")
                wo_t = ld([128, NCH, C], FP8,
                          wo_d.rearrange("p (ko m) -> p ko m", ko=NCH), "wo")

            def stage_a(p, si, sq2, sk2, pk2, ramp_tiles=None, extras=None):
                """One 512-token slab: x DMA, k/q/v projections, silu, pk.

                ramp_tiles: when set (pipeline ramp), the per-chunk scan ops
                are interleaved right behind their producers so the DVE
                starts as soon as the first silu lands."""
                t0 = P2 * p + S * si
                hs = slice(S * si, S * si + S)
                if wk_t is None:
                    load_wk()
                x_t = xpool.tile([128, KCH, S], FP8, tag="x", name=f"x{p}_{si}")
                si_g = 2 * p + si
                nc.sync.dma_start(
                    x_t[:], x_d.rearrange("p (s ko t) -> p s ko t", s=T // S,
                                          ko=KCH)[:, si_g, :, :])
                if wq_t is None:
                    load_weights()
                K2 = KCH // 2
                for c in range(NCH):
                    cs = slice(128 * c, 128 * (c + 1))
                    # k first: silu_k -> pk frees the v PSUM bank in time for
                    # the next chunk's v matmuls (no PE stall with 3 banks)
                    ps_k = ppool.tile([128, S], F32, tag="proj", name=f"psk{p}{si}_{c}")
                    for k in range(K2):
                        nc.tensor.matmul(ps_k[:], wk_t[:, 2 * k:2 * k + 2, cs],
                                         x_t[:, 2 * k:2 * k + 2, :],
                                         start=(k == 0), stop=(k == K2 - 1), perf_mode=DR)
                    nc.scalar.activation(sk2[c][:, hs], ps_k[:], SILU, scale=PS)
                    if ramp_tiles is not None:
                        _, _, _, _, ks2, _, _, _ = ramp_tiles
                        ik = 0.0 if prev_ks[c] is None else \
                            prev_ks[c][0][:, prev_ks[c][1]:prev_ks[c][1] + 1]
                        nc.vector.tensor_tensor_scan(
                            ks2[c][:, hs], sk2[c][:, hs], ones_t[:, :S],
                            initial=ik, op0=ADD, op1=ADD)
                        prev_ks[c] = (ks2[c], S * si + S - 1)
                    ps_q = ppool.tile([128, S], F32, tag="proj", name=f"psq{p}{si}_{c}")
                    for k in range(K2):
                        nc.tensor.matmul(ps_q[:], wq_t[:, 2 * k:2 * k + 2, cs],
                                         x_t[:, 2 * k:2 * k + 2, :],
                                         start=(k == 0), stop=(k == K2 - 1), perf_mode=DR)
                    nc.scalar.activation(sq2[c][:, hs], ps_q[:], SILU,
                                         bias=bq_t[:, c:c + 1], scale=PS)
                    if ramp_tiles is not None:
                        nc.scalar.activation(ramp_tiles[3][c][:, hs],
                                             sq2[c][:, hs], COPY, bias=1.0)
                    ps_v = ppool.tile([128, S], F32, tag="proj", name=f"psv{p}{si}_{c}")
                    # rank-1 fp8-DR ones-row update adds 8*bv into the raw v PSUM
                    nc.tensor.matmul(ps_v[:], bv_t[:, :, cs], onz_t[:, :, :],
                                     start=True, stop=False, perf_mode=DR)
                    for k in range(K2):
                        nc.tensor.matmul(ps_v[:], wv_t[:, 2 * k:2 * k + 2, cs],
                                         x_t[:, 2 * k:2 * k + 2, :],
                                         start=False, stop=(k == K2 - 1), perf_mode=DR)
                    # pk = (silu_k + 1) * (8*v)   [raw 8x scale, fixed via fm]
                    nc.vector.scalar_tensor_tensor(
                        pk2[c][:, hs], sk2[c][:, hs], 1.0, ps_v[:], op0=ADD, op1=MULT)
                    if ramp_tiles is not None:
                        _, _, _, phq2, ks2, kvs2, pq2, nm2 = ramp_tiles
                        ikv = 0.0 if prev_kvs[c] is None else \
                            prev_kvs[c][0][:, prev_kvs[c][1]:prev_kvs[c][1] + 1]
                        nc.vector.tensor_tensor_scan(
                            kvs2[c][:, hs], pk2[c][:, hs], ones_t[:, :S],
                            initial=ikv, op0=ADD, op1=BYPASS)
                        prev_kvs[c] = (kvs2[c], S * si + S - 1)
                        nc.gpsimd.tensor_mul(pq2[c][:, hs], phq2[c][:, hs],
                                             ks2[c][:, hs])
                        nc.gpsimd.tensor_mul(nm2[c][:, hs], phq2[c][:, hs],
                                             kvs2[c][:, hs])
                    if extras is not None:
                        extras(c, si)

            def scan_g(p, h0, nh, tiles, drain=False):
                """phi_q, cumsums, pq, nm on columns [h0*S, (h0+nh)*S)."""
                sq2, sk2, pk2, phq2, ks2, kvs2, pq2, nm2 = tiles
                sl = slice(h0 * S, (h0 + nh) * S)
                for c in range(NCH):
                    nc.vector.tensor_scalar_add(phq2[c][:, sl], sq2[c][:, sl], 1.0)
                    ik = 0.0 if prev_ks[c] is None else \
                        prev_ks[c][0][:, prev_ks[c][1]:prev_ks[c][1] + 1]
                    nc.vector.tensor_tensor_scan(ks2[c][:, sl], sk2[c][:, sl],
                                                 ones_t[:, :nh * S], initial=ik,
                                                 op0=ADD, op1=ADD)
                    ikv = 0.0 if prev_kvs[c] is None else \
                        prev_kvs[c][0][:, prev_kvs[c][1]:prev_kvs[c][1] + 1]
                    nc.vector.tensor_tensor_scan(kvs2[c][:, sl], pk2[c][:, sl],
                                                 ones_t[:, :nh * S], initial=ikv,
                                                 op0=ADD, op1=BYPASS)
                    prev_ks[c] = (ks2[c], (h0 + nh) * S - 1)
                    prev_kvs[c] = (kvs2[c], (h0 + nh) * S - 1)
                    eng = nc.vector if drain else nc.gpsimd
                    eng.tensor_mul(pq2[c][:, sl], phq2[c][:, sl], ks2[c][:, sl])
                    nc.gpsimd.tensor_mul(nm2[c][:, sl], phq2[c][:, sl],
                                         kvs2[c][:, sl])

            rec2_hist = {}

            def tail_a(p, hlist, pq2, drain=False):
                """den -> reciprocal -> bf16 rec for the given halves of pair p."""
                if p not in rec2_hist:
                    rec2_hist[p] = apool.tile([8, P2], BF16, tag="rec2",
                                              name=f"rec2{p}")
                rec2 = rec2_hist[p]
                for h in hlist:
                    hs = slice(S * h, S * h + S)
                    den_ps = dpool.tile([8, S], F32, tag="den", name=f"den{p}_{h}")
                    for c in range(NCH):
                        nc.tensor.matmul(den_ps[:], em_t[:, c, :], pq2[c][:, hs],
                                         start=(c == 0), stop=(c == NCH - 1))
                    rec32 = apool.tile([8, S], F32, tag="rec32", name=f"rec32{p}_{h}")
                    nc.vector.reciprocal_approx_fast(rec32[:], den_ps[:])
                    if drain:
                        # cast on DVE: keeps the recip->rb chain off ACT's queue
                        nc.vector.tensor_copy(rec2[:, hs], rec32[:])
                    else:
                        nc.scalar.copy(rec2[:, hs], rec32[:])
                return rec2

            ot_hist = {}
            at_hist = {}

            def tail_b1_one(p, c, h, nm2, rec2):
                if p not in at_hist:
                    at_hist[p] = [outpool.tile([128, 2, P2], FP8, tag=f"at{kk}",
                                               name=f"at{p}_{kk}")
                                  for kk in range(NCH // 2)]
                at_l = at_hist[p]
                hs = slice(S * h, S * h + S)
                cs = slice(128 * c, 128 * (c + 1))
                rb = bpool.tile([128, S], F32, tag="rb", name=f"rb{p}_{c}{h}")
                nc.tensor.matmul(rb[:], fm_t[:, cs], rec2[:, hs],
                                 start=True, stop=True)
                nc.vector.tensor_mul(at_l[c // 2][:, c % 2, hs],
                                     nm2[c][:, hs], rb[:])

            def tail_b1(p, hlist, nm2, rec2):
                """broadcast -> attn (fp8)."""
                for h in hlist:
                    for c in range(NCH):
                        tail_b1_one(p, c, h, nm2, rec2)

            def tail_b2(p, hlist, drain=False):
                """Wo matmuls -> bf16 out copies."""
                at_l = at_hist[p]
                for h in hlist:
                    hs = slice(S * h, S * h + S)
                    for moo in range(MO // 2):
                        ot = otpool.tile([128, 2, S], BF16, tag=f"ot{moo}",
                                         name=f"ot{p}_{h}_{moo}")
                        for mo2 in range(2):
                            mo = 2 * moo + mo2
                            ms = slice(128 * mo, 128 * (mo + 1))
                            wo_ps = opool.tile([128, S], F32, tag="wo",
                                               name=f"wo{p}_{h}_{mo}")
                            for kk in range(NCH // 2):
                                nc.tensor.matmul(
                                    wo_ps[:], wo_t[:, 2 * kk:2 * kk + 2, ms],
                                    at_l[kk][:, :, hs],
                                    start=(kk == 0), stop=(kk == NCH // 2 - 1),
                                    perf_mode=DR)
                            if drain and mo2 == 1 and moo % 2 == 0:
                                nc.vector.tensor_copy(ot[:, mo2, :], wo_ps[:])
                            else:
                                nc.scalar.copy(ot[:, mo2, :], wo_ps[:])
                        ot_hist[(p, h, moo)] = ot

            def out_dma(p, hlist=(0, 1)):
                """Issue the 8 out DMAs for pair p from SP, a full pair after
                tail_b(p): every wait is satisfied at issue time, so the
                in-order SP DGE queue never blocks x loads. Destination is
                slab-major per partition: each descriptor is one contiguous
                2KB run (host un-permutes in assemble())."""
                o5 = out_d.rearrange("p (s moo mo2 t) -> p s moo mo2 t",
                                     s=T // S, moo=MO // 2, mo2=2)
                for h in hlist:
                    sg = 2 * p + h
                    for moo in range(MO // 2):
                        ot = ot_hist.pop((p, h, moo))
                        nc.sync.dma_start(o5[:, sg, moo, :, :], ot[:])

            def alloc_pair(p):
                def mk(pool, tag):
                    return [pool.tile([128, P2], BF16, tag=f"{tag}{c}",
                                      name=f"{tag}{p}_{c}")
                            for c in range(NCH)]
                return (mk(apool, "sq"), mk(apool, "sk"), mk(apool, "pk"),
                        mk(apool, "phq"), mk(spool, "ks"), mk(spool, "kvs"),
                        mk(apool, "pq"), mk(apool, "nm"))

            hist = {}
            for p in range(NPAIR):
                if p >= 3:
                    out_dma(p - 3)
                tiles = alloc_pair(p)
                sq2, sk2, pk2 = tiles[0], tiles[1], tiles[2]
                pq2, nm2 = tiles[6], tiles[7]
                if p == 0:
                    # ramp: per-chunk interleaved scans so downstream starts early
                    stage_a(p, 0, sq2, sk2, pk2, ramp_tiles=tiles)
                    stage_a(p, 1, sq2, sk2, pk2, ramp_tiles=tiles)
                elif p < NPAIR - 1:
                    stage_a(p, 0, sq2, sk2, pk2, ramp_tiles=tiles)
                    stage_a(p, 1, sq2, sk2, pk2, ramp_tiles=tiles)
                    pq2_o, nm2_o = hist.pop(p - 1)
                    rec2_o = tail_a(p - 1, [0, 1], pq2_o)
                    hist[p - 1] = (nm2_o, rec2_o)
                    if p >= 2:
                        nm2_o, rec2_o = hist[p - 2]
                        tail_b1(p - 2, [0, 1], nm2_o, rec2_o)
                        tail_b2(p - 2, [0, 1])
                else:
                    # drain: 512-granular scans/tails for the last pair
                    stage_a(p, 0, sq2, sk2, pk2)
                    scan_g(p, 0, 1, tiles)
                    stage_a(p, 1, sq2, sk2, pk2)
                    nm2_o, rec2_o = hist[p - 2]
                    tail_b1(p - 2, [0, 1], nm2_o, rec2_o)
                    tail_b2(p - 2, [0, 1])
                    scan_g(p, 1, 1, tiles, drain=True)
                    pq2_o, nm2_o = hist.pop(p - 1)
                    rec2_o = tail_a(p - 1, [0, 1], pq2_o)
                    hist[p - 1] = (nm2_o, rec2_o)
                hist[p] = (pq2, nm2)

            # drain
            out_dma(1)
            nm2_o, rec2_o = hist.pop(NPAIR - 2)
            tail_b1(NPAIR - 2, [0, 1], nm2_o, rec2_o)
            tail_b2(NPAIR - 2, [0, 1], drain=True)
            pq2_l, nm2_l = hist.pop(NPAIR - 1)
            rec2_l = tail_a(NPAIR - 1, [0], pq2_l, drain=True)
            out_dma(NPAIR - 2)
            tail_b1(NPAIR - 1, [0], nm2_l, rec2_l)
            tail_b2(NPAIR - 1, [0], drain=True)
            tail_a(NPAIR - 1, [1], pq2_l, drain=True)
            out_dma(NPAIR - 1, (0,))
            tail_b1(NPAIR - 1, [1], nm2_l, rec2_l)
            tail_b2(NPAIR - 1, [1], drain=True)
            out_dma(NPAIR - 1, (1,))

    nc.compile()
    return nc


_NC_CACHE = {}


def _get_nc():
    if "nc" not in _NC_CACHE:
        _NC_CACHE["nc"] = build()
    return _NC_CACHE["nc"]


def _wpm(w):
    """[(ko ki), m] -> partition-major [ki, (ko m)] so DMA lines are 4KB."""
    ko = w.shape[0] // 128
    return np.ascontiguousarray(
        w.reshape(ko, 128, w.shape[1]).transpose(1, 0, 2).reshape(128, -1))


def _xpm(xb):
    """[(ko ki), t] -> [ki, (slab ko t)]: one contiguous 4KB run per slab."""
    x4 = xb.reshape(KCH, 128, T // S, S)
    return np.ascontiguousarray(
        x4.transpose(1, 2, 0, 3).reshape(128, -1))


def make_in_maps(x, Wq, bq, Wk, Wv, bv, Wo, bo):
    bf = ml_dtypes.bfloat16
    f8 = ml_dtypes.float8_e4m3
    WS = 1.0 / (XS * PS)  # weight pre-scale so that psum * PS = W @ x exactly
    x3 = np.asarray(x, np.float32)[..., 0]                      # (B, C, T)
    E = np.zeros((CG, 8), np.float32)
    for ch in range(CG):
        E[ch, ch // DH] = 1.0
    em = np.ascontiguousarray(E.reshape(NCH, 128, 8).transpose(1, 0, 2))
    ones = np.ones((128, P2), bf)
    in_maps = []
    for core in range(8):
        b, g = core // 2, core % 2
        sl = slice(CG * g, CG * (g + 1))
        in_maps.append({
            "x": np.clip(_xpm(x3[b]) * XS, -240, 240).astype(f8),
            "wq": np.clip(_wpm(np.asarray(Wq, np.float32)[sl, :].T) * WS, -240, 240).astype(f8),
            "wk": np.clip(_wpm(np.asarray(Wk, np.float32)[sl, :].T) * WS, -240, 240).astype(f8),
            "wv": np.clip(_wpm(np.asarray(Wv, np.float32)[sl, :].T) * WS, -240, 240).astype(f8),
            "wo": np.clip(_wpm(np.asarray(Wo, np.float32)[:, sl].T) * 8.0, -240, 240).astype(f8),
            "bq": np.ascontiguousarray(np.asarray(bq, np.float32)[sl].reshape(NCH, 128).T),
            "bv8": np.clip(np.stack([np.asarray(bv, np.float32)[sl] * 8.0,
                                     np.zeros(CG, np.float32)]), -240, 240).reshape(1, 2 * CG).astype(f8),
            "emat": em.astype(bf),
            "fmat": np.ascontiguousarray(E.T / 8.0).astype(bf),
            "ones": ones,
            "onz": np.stack([np.ones(S, np.float32),
                             np.zeros(S, np.float32)]).reshape(1, 2 * S).astype(f8),
        })
    return in_maps


def _opm(o):
    """[ki, (s moo mo2 t)] -> [(moo mo2 ki), (s t)] = [C, T]."""
    o5 = o.reshape(128, T // S, MO // 2, 2, S)
    return o5.transpose(2, 3, 0, 1, 4).reshape(C, T)


def assemble(results, bo):
    out = np.empty((B, C, T, 1), np.float32)
    bo_f = np.asarray(bo, np.float32)[:, None]
    for b in range(B):
        p0 = _opm(np.asarray(results[2 * b]["out"], np.float32))
        p1 = _opm(np.asarray(results[2 * b + 1]["out"], np.float32))
        out[b, :, :, 0] = (p0 + p1) * (1.0 / 64.0) + bo_f
    return out


def kernel(x, Wq, bq, Wk, Wv, bv, Wo, bo):
    nc = _get_nc()
    in_maps = make_in_maps(x, Wq, bq, Wk, Wv, bv, Wo, bo)
    res = run_bass_kernel_spmd(nc, in_maps, core_ids=list(range(8)))
    return assemble(res.results, bo)
